# revision 14
# baseline (speedup 1.0000x reference)
"""Trainium2 Bass kernel for GroupNorm + multi-head self-attention block.

Reference computation (per batch element):
    xn  = GroupNorm(x; 32 groups, eps=1e-5) * norm_w + norm_b
    qkv = qkv_w @ xn + qkv_b          (1x1 conv == channel matmul)
    q,k,v split; 4 heads of dh=128 over 1024 spatial positions
    attn = softmax(q^T k * C**-0.5); out = attn @ v
    out = proj_w @ out + proj_b + xn

Sharding: pure data-parallel over batch (16 batches / 8 cores = 2 per core),
no collectives.

Precision strategy (tolerance 2e-2; fp8 errors land on the attention path,
which is only ~3.5% of the output norm):
  - All large matmuls in fp8e4m3 with DoubleRow perf mode (2 k-subtiles of
    128 contracted per instruction at 0.5 cycles/row).
  - Weights are prescaled x16 on the host so fp8 quantization stays in the
    normal range; the inverse scales fold into (free) evac scale factors.
  - Softmax: exp runs split across the Scalar engine (native Exp, fp8 out)
    and the Vector engine (Schraudolph bit-trick: affine + truncating
    convert to uint8, bitcast as fp8e4m3).
  - Bias folding: v-bias folds into proj bias (host), k-bias and qb.kb
    cancel under softmax over keys, qb.k is computed by tiny DoubleRow
    matmuls in the v orientation and applied as the per-partition exp bias.
  - GroupNorm statistics in fp32; residual in bf16; final output f32.

Score softmax runs over the key axis j, which sits on PSUM partitions, so
the denominator is a DoubleRow matmul against a constant 1/16 tile (the
1/16 makes rc = 16/sum(e), boosting attnout into fp8's sweet range; the
16 folds back out in the proj evac).
"""

from contextlib import ExitStack

import numpy as np

B = 16          # full batch
C = 512         # channels
S = 1024        # spatial (32*32)
HEADS = 4
DH = C // HEADS         # 128, head dim == partition tile
GROUPS = 32
EPS = 1e-5
NCORES = 8
BPC = B // NCORES       # 2 batches per core
CT = C // 128           # 4 channel tiles
SCALE = float(C) ** -0.5
JT = S // 128           # 8 j-tiles (key positions)
NH = S // 512           # 2 free-dim halves
LOG2E = 1.4426950408889634
A_SCH = 8.0 * LOG2E            # fp8e4m3 bits per unit exp-argument
B_SCH = 56.05                  # calibrated for truncating convert
DVE_JT = (2, 4, 6)             # j-tiles whose exp runs on the Vector engine

_CACHE = {}


def _emit(tc, io):
    from concourse import mybir

    nc = tc.nc
    f32 = mybir.dt.float32
    f32r = mybir.dt.float32r
    bf16 = mybir.dt.bfloat16
    f8 = mybir.dt.float8e4
    u8 = mybir.dt.uint8
    Act = mybir.ActivationFunctionType
    Alu = mybir.AluOpType
    PM = mybir.MatmulPerfMode

    x_d = io["x"]
    out_d = io["out"]

    with ExitStack() as ctx:
        consts = ctx.enter_context(tc.tile_pool(name="consts", bufs=1))
        x_pool = ctx.enter_context(tc.tile_pool(name="x_pool", bufs=6))
        xn_pool = ctx.enter_context(tc.tile_pool(name="xn_pool", bufs=1))
        stats = ctx.enter_context(tc.tile_pool(name="stats", bufs=4))
        qk_pool = ctx.enter_context(tc.tile_pool(name="qk_pool", bufs=2))
        qs_pool = ctx.enter_context(tc.tile_pool(name="qs_pool", bufs=2))
        vt_pool = ctx.enter_context(tc.tile_pool(name="vt_pool", bufs=2))
        ao_pool = ctx.enter_context(tc.tile_pool(name="ao_pool", bufs=2))
        e_pool = ctx.enter_context(tc.tile_pool(name="e_pool", bufs=4))
        rc_pool = ctx.enter_context(tc.tile_pool(name="rc_pool", bufs=4))
        ob_pool = ctx.enter_context(tc.tile_pool(name="ob_pool", bufs=4))
        fo_pool = ctx.enter_context(tc.tile_pool(name="fo_pool", bufs=3))
        b8_pool = ctx.enter_context(tc.tile_pool(name="b8_pool", bufs=2))
        # PSUM: big(2 x 2 banks) + dn(2 x 1) + ot(2 x 1) = 8 banks
        mm = ctx.enter_context(tc.tile_pool(name="mm", bufs=2, space="PSUM"))
        dn_ps = ctx.enter_context(tc.tile_pool(name="dn_ps", bufs=2, space="PSUM"))
        o_ps = ctx.enter_context(tc.tile_pool(name="o_ps", bufs=2, space="PSUM"))

        # ---- constants ----
        qkvT8 = consts.tile([128, CT, 3 * C], f8, name="qkvT8")
        nc.sync.dma_start(out=qkvT8, in_=io["qkvT8"])
        projT8 = consts.tile([128, CT, C], f8, name="projT8")
        nc.sync.dma_start(out=projT8, in_=io["projT8"])
        wstar8 = consts.tile([128, CT, HEADS], f8, name="wstar8")
        nc.sync.dma_start(out=wstar8, in_=io["wstar8"])
        gnw_sb = consts.tile([128, CT], f32, name="gnw_sb")
        nc.sync.dma_start(out=gnw_sb, in_=io["gnw"])
        gnb_sb = consts.tile([128, CT], f32, name="gnb_sb")
        nc.sync.dma_start(out=gnb_sb, in_=io["gnb"])
        projb_sb = consts.tile([128, CT], f32, name="projb_sb")
        nc.sync.dma_start(out=projb_sb, in_=io["projb"])
        indp_sb = consts.tile([128, 8], f32r, name="indp_sb")
        nc.sync.dma_start(out=indp_sb, in_=io["indp"])
        indb_sb = consts.tile([8, 128], f32r, name="indb_sb")
        nc.sync.dma_start(out=indb_sb, in_=io["indb"])
        ones8 = consts.tile([128, 2, 128], f8, name="ones8")
        nc.vector.memset(ones8, 1.0 / 16.0)
        eps_sb = consts.tile([8, 1], f32, name="eps_sb")
        nc.vector.memset(eps_sb, EPS)

        # normalized x, both batches: bf16 residual + fp8 matmul operand
        xn_bf = xn_pool.tile([128, CT, BPC, S], bf16, name="xn_bf")
        xn_f8 = xn_pool.tile([128, CT, BPC, S], f8, name="xn_f8")

        gn_state = {}

        def emit_gn_stats(b):
            """GroupNorm per-channel stats for batch b (Vector engine only)."""
            for k in range(CT):
                xt = x_pool.tile([128, S], f32, name="xt")
                nc.sync.dma_start(out=xt, in_=x_d[b, k * 128:(k + 1) * 128, :])
                sb_stf = stats.tile([128, 4], f32, name="sb_stf")
                sb_st = stats.tile([128, 4], f32r, name="sb_st")
                bn6 = stats.tile([128, 2, 6], f32, name="bn6")
                for u in range(2):
                    nc.vector.bn_stats(
                        out=bn6[:, u, :], in_=xt[:, u * 512:(u + 1) * 512]
                    )
                nc.vector.bn_aggr(out=sb_stf[:, 0:2], in_=bn6)
                nc.vector.tensor_mul(sb_stf[:, 2:3], sb_stf[:, 0:1], sb_stf[:, 0:1])
                nc.vector.tensor_copy(out=sb_stf[:, 3:4], in_=sb_stf[:, 0:1])
                nc.vector.tensor_copy(out=sb_st, in_=sb_stf)
                gn_state[(b, k)] = (xt, sb_st)

        def emit_gn_rest(b):
            """Group pooling + broadcast + normalize for batch b."""
            for k in range(CT):
                xt, sb_st = gn_state.pop((b, k))
                # pool over 16-channel groups (x 1/16): pg[g, {mean,var,mean2}]
                pgt = mm.tile([128, 1024], f32, name="mm", tag="mm")
                pg = pgt[0:8, 0:4]
                nc.tensor.matmul(pg, lhsT=indp_sb, rhs=sb_st, start=True, stop=True)
                pgs = stats.tile([8, 4], f32, name="pgs")
                nc.vector.tensor_copy(out=pgs, in_=pg)
                g_sb = stats.tile([8, 2], f32r, name="g_sb")
                tmp8 = stats.tile([8, 2], f32, name="tmp8")
                nc.vector.tensor_copy(out=g_sb[:, 0:1], in_=pgs[:, 0:1])
                nc.vector.tensor_mul(tmp8[:, 0:1], pgs[:, 0:1], pgs[:, 0:1])
                nc.vector.tensor_add(tmp8[:, 1:2], pgs[:, 1:2], pgs[:, 2:3])
                nc.vector.tensor_sub(tmp8[:, 1:2], tmp8[:, 1:2], tmp8[:, 0:1])
                nc.scalar.activation(
                    out=g_sb[:, 1:2], in_=tmp8[:, 1:2], func=Act.Sqrt, bias=eps_sb
                )
                with nc.allow_low_precision("fp22 matmul input rounding"):
                    nc.vector.reciprocal(out=g_sb[:, 1:2], in_=g_sb[:, 1:2])
                # broadcast group stats to channels: bc [128, {mean, rstd}]
                bct = mm.tile([128, 1024], f32, name="mm", tag="mm")
                bc = bct[:, 0:2]
                nc.tensor.matmul(bc, lhsT=indb_sb, rhs=g_sb, start=True, stop=True)
                # sc cols: [posbias, scale];  xn = x*scale + posbias
                sc = stats.tile([128, 2], f32, name="sc")
                nc.vector.tensor_scalar_mul(sc[:, 1:2], bc[:, 1:2], gnw_sb[:, k:k + 1])
                nc.vector.tensor_mul(sc[:, 0:1], bc[:, 0:1], sc[:, 1:2])
                nc.vector.tensor_scalar(
                    sc[:, 0:1], sc[:, 0:1], gnb_sb[:, k:k + 1], None, op0=Alu.subtract
                )
                nc.vector.tensor_scalar_mul(sc[:, 0:1], sc[:, 0:1], -1.0)
                # normalize + residual copy on GpSimd (SBUF -> SBUF only)
                nc.gpsimd.tensor_scalar(
                    xn_bf[:, k, b, :], xt, sc[:, 1:2], sc[:, 0:1],
                    op0=Alu.mult, op1=Alu.add,
                )
                nc.gpsimd.tensor_copy(
                    out=xn_f8[:, k, b, :], in_=xn_bf[:, k, b, :]
                )

        qs_sb = {}
        ks_sb = {}
        vt_sb = {}
        ao_sb = {}
        b8_sb = {}
        b8s_sb = {}

        def emit_qkv(b):
            # qb.k bias: tiny DoubleRow matmuls in v orientation -> [j, jt, h]
            psb_t = mm.tile([128, 1024], f32, name="mm", tag="mm")
            psb = psb_t[:, 0:JT * HEADS]
            for jt in range(JT):
                for u in range(2):
                    nc.tensor.matmul(
                        psb[:, jt * HEADS:(jt + 1) * HEADS],
                        lhsT=xn_f8[:, 2 * u:2 * u + 2, b, jt * 128:(jt + 1) * 128],
                        rhs=wstar8[:, 2 * u:2 * u + 2, :],
                        start=(u == 0), stop=(u == 1),
                        perf_mode=PM.DoubleRow,
                    )
            b8_sb[b] = b8_pool.tile([128, JT, HEADS], f32, name="b8_sb")
            nc.scalar.activation(
                out=b8_sb[b], in_=psb, func=Act.Identity, scale=SCALE / 256.0
            )
            b8s_sb[b] = b8_pool.tile([128, JT, HEADS], f32, name="b8s_sb")
            nc.gpsimd.tensor_scalar(
                b8s_sb[b], b8_sb[b], A_SCH, B_SCH, op0=Alu.mult, op1=Alu.add
            )

            # q, k: [128(dh), head, 1024] fp8, natural layout
            q8 = qk_pool.tile([128, HEADS, S], f8, name="q8")
            k8 = qk_pool.tile([128, HEADS, S], f8, name="k8")
            for m in range(2 * HEADS):
                dst = q8 if m < HEADS else k8
                ps = mm.tile([128, 1024], f32, name="mm", tag="mm")
                for n in range(NH):
                    for u in range(2):
                        nc.tensor.matmul(
                            ps[:, n * 512:(n + 1) * 512],
                            lhsT=qkvT8[:, 2 * u:2 * u + 2, m * 128:(m + 1) * 128],
                            rhs=xn_f8[:, 2 * u:2 * u + 2, b, n * 512:(n + 1) * 512],
                            start=(u == 0), stop=(u == 1),
                            perf_mode=PM.DoubleRow,
                        )
                nc.scalar.activation(
                    out=dst[:, m % HEADS, :], in_=ps, func=Act.Copy,
                    scale=1.0 / 16.0,
                )
            # v^T: [128(j), jt, 512(c_v)] fp8
            vt_sb[b] = vt_pool.tile([128, JT, C], f8, name="vt_sb")
            for t in range(JT // 2):
                ps = mm.tile([128, 1024], f32, name="mm", tag="mm")
                for s_ in range(2):
                    for u in range(2):
                        jt = 2 * t + s_
                        nc.tensor.matmul(
                            ps[:, s_ * 512:(s_ + 1) * 512],
                            lhsT=xn_f8[:, 2 * u:2 * u + 2, b, jt * 128:(jt + 1) * 128],
                            rhs=qkvT8[:, 2 * u:2 * u + 2, 2 * C:3 * C],
                            start=(u == 0), stop=(u == 1),
                            perf_mode=PM.DoubleRow,
                        )
                nc.scalar.activation(
                    out=vt_sb[b][:, 2 * t:2 * t + 2, :], in_=ps, func=Act.Copy,
                    scale=1.0 / 16.0,
                )
            # shuffle q,k into the split-dh layout for DoubleRow scores:
            # qs[p, t, h, i] = q[p + 64 t, h, i], partitions 0..63
            qs_sb[b] = qs_pool.tile([64, 2, HEADS, S], f8, name="qs_sb")
            ks_sb[b] = qs_pool.tile([64, 2, HEADS, S], f8, name="ks_sb")
            for src, dst in ((q8, qs_sb[b]), (k8, ks_sb[b])):
                nc.sync.dma_start(out=dst[:, 0, :, :], in_=src[0:64, :, :])
                nc.sync.dma_start(out=dst[:, 1, :, :], in_=src[64:128, :, :])

        def emit_attn(b):
            ao_sb[b] = ao_pool.tile([128, HEADS, S], f8, name="ao_sb")
            for h in range(HEADS):
                dns = [dn_ps.tile([128, 512], f32, name="dn") for _ in range(NH)]
                ots = [o_ps.tile([128, 512], f32, name="ot") for _ in range(NH)]
                e2s = [e_pool.tile([128, 2, S], f8, name="e2") for _ in range(JT // 2)]

                def dn_ot(t, last):
                    for n in range(NH):
                        lo, hi = n * 512, (n + 1) * 512
                        nc.tensor.matmul(
                            dns[n], lhsT=ones8, rhs=e2s[t][:, :, lo:hi],
                            start=(t == 0), stop=last,
                            perf_mode=PM.DoubleRow,
                        )
                        nc.tensor.matmul(
                            ots[n],
                            lhsT=vt_sb[b][:, 2 * t:2 * t + 2, h * 128:(h + 1) * 128],
                            rhs=e2s[t][:, :, lo:hi],
                            start=(t == 0), stop=last,
                            perf_mode=PM.DoubleRow,
                        )
                    if last:
                        for n in range(NH):
                            lo, hi = n * 512, (n + 1) * 512
                            rc = rc_pool.tile([128, 512], f32, name="rc")
                            nc.vector.reciprocal_approx_fast(out=rc, in_=dns[n])
                            if b == 0 and h == 0 and "d_rc" in io:
                                nc.sync.dma_start(out=io["d_rc"][n], in_=rc)
                            ob = ob_pool.tile([128, 512], bf16, name="ob")
                            nc.scalar.copy(out=ob, in_=ots[n])
                            nc.gpsimd.tensor_mul(
                                ao_sb[b][:, h, lo:hi], ob, rc
                            )

                # scores + exp run one j-tile pair ahead of dn/ot accumulation
                for jt in range(JT):
                    sp = mm.tile([128, 1024], f32, name="mm", tag="mm")
                    for n in range(NH):
                        lo, hi = n * 512, (n + 1) * 512
                        nc.tensor.matmul(
                            sp[:, lo:hi],
                            lhsT=ks_sb[b][:, :, h, jt * 128:(jt + 1) * 128],
                            rhs=qs_sb[b][:, :, h, lo:hi],
                            start=True, stop=True,
                            perf_mode=PM.DoubleRow,
                        )
                    if jt in DVE_JT:
                        nc.vector.tensor_scalar(
                            e2s[jt // 2].bitcast(u8)[:, jt % 2, :], sp,
                            SCALE * A_SCH, b8s_sb[b][:, jt, h:h + 1],
                            op0=Alu.mult, op1=Alu.add,
                        )
                    else:
                        nc.scalar.activation(
                            out=e2s[jt // 2][:, jt % 2, :], in_=sp,
                            func=Act.Exp,
                            scale=SCALE, bias=b8_sb[b][:, jt, h:h + 1],
                        )
                    if jt % 2 == 1:
                        if b == 0 and h == 0 and "d_e" in io:
                            nc.sync.dma_start(
                                out=io["d_e"][jt // 2], in_=e2s[jt // 2]
                            )
                        if 3 <= jt < JT - 1:
                            dn_ot((jt - 3) // 2, last=False)
                dn_ot(2, last=False)
                dn_ot(3, last=True)

        def emit_proj(b):
            for m in range(CT):
                ps = mm.tile([128, 1024], f32, name="mm", tag="mm")
                for n in range(NH):
                    for u in range(2):
                        nc.tensor.matmul(
                            ps[:, n * 512:(n + 1) * 512],
                            lhsT=projT8[:, 2 * u:2 * u + 2, m * 128:(m + 1) * 128],
                            rhs=ao_sb[b][:, 2 * u:2 * u + 2, n * 512:(n + 1) * 512],
                            start=(u == 0), stop=(u == 1),
                            perf_mode=PM.DoubleRow,
                        )
                fo = fo_pool.tile([128, S], f32, name="fo")
                # fo = (ps/256 + projb_eff) + xn
                nc.vector.affine_then_add(
                    out=fo,
                    in0=ps,
                    in1=xn_bf[:, m, b, :],
                    scale=1.0 / 256.0,
                    bias=projb_sb[:, m:m + 1],
                )
                nc.sync.dma_start(
                    out=out_d[b, m * 128:(m + 1) * 128, :], in_=fo,
                )

        emit_gn_stats(0)
        emit_gn_rest(0)
        emit_gn_stats(1)
        emit_qkv(0)
        emit_gn_rest(1)
        emit_attn(0)
        emit_qkv(1)
        emit_proj(0)
        emit_attn(1)
        emit_proj(1)


def _build_nc():
    import concourse.tile as tile
    from concourse import bacc, mybir

    f32 = mybir.dt.float32
    f32r = mybir.dt.float32r
    f8 = mybir.dt.float8e4
    nc = bacc.Bacc("TRN2", target_bir_lowering=False, debug=False)
    io = {
        "x": nc.dram_tensor("x", [BPC, C, S], f32, kind="ExternalInput").ap(),
        "qkvT8": nc.dram_tensor("qkvT8", [128, CT, 3 * C], f8, kind="ExternalInput").ap(),
        "projT8": nc.dram_tensor("projT8", [128, CT, C], f8, kind="ExternalInput").ap(),
        "wstar8": nc.dram_tensor("wstar8", [128, CT, HEADS], f8, kind="ExternalInput").ap(),
        "gnw": nc.dram_tensor("gnw", [128, CT], f32, kind="ExternalInput").ap(),
        "gnb": nc.dram_tensor("gnb", [128, CT], f32, kind="ExternalInput").ap(),
        "projb": nc.dram_tensor("projb", [128, CT], f32, kind="ExternalInput").ap(),
        "indp": nc.dram_tensor("indp", [128, 8], f32r, kind="ExternalInput").ap(),
        "indb": nc.dram_tensor("indb", [8, 128], f32r, kind="ExternalInput").ap(),
        "out": nc.dram_tensor("out", [BPC, C, S], f32, kind="ExternalOutput").ap(),
    }
    with tile.TileContext(nc) as tc:
        _emit(tc, io)
    nc.compile()
    return nc


def get_nc():
    if "nc" not in _CACHE:
        _CACHE["nc"] = _build_nc()
    return _CACHE["nc"]


def make_const_inputs(norm_w, norm_b, qkv_w, qkv_b, proj_w, proj_b):
    """Host-side constant tensors shared by all cores."""
    import ml_dtypes

    f = np.float32
    fp8 = ml_dtypes.float8_e4m3
    qkv_w = np.asarray(qkv_w, dtype=np.float64)
    qkv_b = np.asarray(qkv_b, dtype=np.float64)
    proj_w = np.asarray(proj_w, dtype=np.float64)
    proj_b = np.asarray(proj_b, dtype=np.float64)

    # qkvT8[p, kt, o] = 16 * qkv_w[o, kt*128 + p]
    qkvT8 = np.ascontiguousarray(
        (16.0 * qkv_w.T).reshape(CT, 128, 3 * C).transpose(1, 0, 2).astype(fp8)
    )
    projT8 = np.ascontiguousarray(
        (16.0 * proj_w.T).reshape(CT, 128, C).transpose(1, 0, 2).astype(fp8)
    )
    # wstar[c, h] = 256 * sum_d qb_h[d] * qkv_w[C + h*128 + d, c]
    wstar = np.stack(
        [
            256.0 * (qkv_b[C + h * 128:C + (h + 1) * 128]
                     @ qkv_w[C + h * 128:C + (h + 1) * 128, :])
            for h in range(HEADS)
        ],
        axis=1,
    )  # [C, HEADS]
    wstar8 = np.ascontiguousarray(
        wstar.reshape(CT, 128, HEADS).transpose(1, 0, 2).astype(fp8)
    )
    # proj bias with folded v-bias: proj_b + proj_w @ qkv_b[2C:3C]
    projb_eff = proj_b + proj_w @ qkv_b[2 * C:3 * C]
    projb = np.ascontiguousarray(projb_eff.reshape(CT, 128).T, dtype=f)
    gnw = np.ascontiguousarray(np.asarray(norm_w).reshape(CT, 128).T, dtype=f)
    gnb = np.ascontiguousarray(np.asarray(norm_b).reshape(CT, 128).T, dtype=f)
    indp = np.zeros((128, 8), dtype=f)
    for p in range(128):
        indp[p, p // 16] = 1.0 / 16.0
    indb = np.zeros((8, 128), dtype=f)
    for p in range(128):
        indb[p // 16, p] = 1.0
    return {
        "qkvT8": qkvT8, "projT8": projT8, "wstar8": wstar8,
        "projb": projb, "gnw": gnw, "gnb": gnb,
        "indp": indp, "indb": indb,
    }


def kernel(x, norm_w, norm_b, qkv_w, qkv_b, proj_w, proj_b, _trace=False):
    from concourse.bass_utils import run_bass_kernel_spmd

    b, c, h, w = x.shape
    assert (b, c, h * w) == (B, C, S), f"unexpected input shape {x.shape}"
    consts = make_const_inputs(norm_w, norm_b, qkv_w, qkv_b, proj_w, proj_b)
    xf = np.ascontiguousarray(x.reshape(B, C, S), dtype=np.float32)
    in_maps = [
        {"x": np.ascontiguousarray(xf[i * BPC:(i + 1) * BPC]), **consts}
        for i in range(NCORES)
    ]
    nc = get_nc()
    res = run_bass_kernel_spmd(
        nc, in_maps, core_ids=list(range(NCORES)), trace=_trace
    )
    out = np.concatenate([r["out"] for r in res.results], axis=0)
    out = out.reshape(B, C, h, w).astype(np.float32)
    if _trace:
        _CACHE["last_results"] = res
    return out
